# revision 8
# baseline (speedup 1.0000x reference)
"""BalanceLoss (BCE + OHEM top-k negatives) on 8 trn2 NeuronCores.

Strategy
--------
Data-parallel: the 32x1x640x640 inputs are flattened and split into 8 equal
shards (one per core).  Each core computes four partial sums over its shard:

    sw = sum(gt * mask)                      (positive count)
    sn = sum((1 - gt) * mask)                (negative count)
    sa = sum(gt * mask * ln(pred))           (-pos_loss_sum)
    sb = sum((1 - gt) * mask * ln(1 - pred)) (-neg_loss_sum over ALL negatives)

On the host the shards are merged.  The OHEM top-k reduces to the full
negative sum whenever k = min(sn, 3*sw) == sn (all negatives kept), which is
the regime for this data distribution; an exact host fallback handles k < sn.

Per-core kernel: ScalarE does the two Ln passes (Ln(pred), Ln(1-pred) via the
free affine scale=-1/bias=1), VectorE does four fused
tensor_tensor_reduce passes (product + free-dim reduction in one
instruction), everything overlapped with the HBM DMA stream.
"""

import os
import sys

import numpy as np

# ---------------------------------------------------------------- constants
FULL_SHAPE = (32, 1, 640, 640)
TOT = 32 * 640 * 640          # 13_107_200 elements
N_CORES = 8
PER_CORE = TOT // N_CORES     # 1_638_400
P = 128                       # SBUF partitions
W = PER_CORE // P             # 12_800 free-dim elements per partition
NT = 8                        # tiles per core
F = W // NT                   # 1_600 free-dim elements per tile
IO_BUFS = 5
TMP_BUFS = 3
NEG_RATIO = 3.0
EPS = 1e-6

_CONCOURSE_PATHS = ("/opt/trn_rl_repo", "/root/.axon_site/_ro/trn_rl_repo")


def _ensure_concourse():
    try:
        import concourse.bass  # noqa: F401
    except ImportError:
        for p in _CONCOURSE_PATHS:
            if os.path.isdir(p) and p not in sys.path:
                sys.path.insert(0, p)
        import concourse.bass  # noqa: F401


_NC_CACHE = {}


def _build_nc(reps=1, nt=None, io_bufs=None, tmp_bufs=None):
    """Build the per-core Bass program (same program on every core).

    reps > 1 unrolls the whole tile loop `reps` times inside one NEFF for
    benchmarking; the fused accum writes are idempotent so results are
    unchanged."""
    nt = NT if nt is None else nt
    io_bufs = IO_BUFS if io_bufs is None else io_bufs
    tmp_bufs = TMP_BUFS if tmp_bufs is None else tmp_bufs
    f = W // nt
    assert W % nt == 0
    key = (reps, nt, io_bufs, tmp_bufs)
    if key in _NC_CACHE:
        return _NC_CACHE[key]
    _ensure_concourse()
    import concourse.bacc as bacc
    import concourse.bass as bass
    import concourse.mybir as mybir
    import concourse.tile as tile

    f32 = mybir.dt.float32
    Act = mybir.ActivationFunctionType
    Alu = mybir.AluOpType

    nc = bacc.Bacc(None, target_bir_lowering=False)
    predD = nc.declare_dram_parameter("pred", [P, W], f32, isOutput=False)
    gtD = nc.declare_dram_parameter("gt", [P, W], f32, isOutput=False)
    maskD = nc.declare_dram_parameter("mask", [P, W], f32, isOutput=False)
    # stats columns: [0:NT]=sum(w) [NT:2NT]=sum(n) [2NT:3NT]=sum(w*l1) [3NT:4NT]=sum(n*l2)
    outD = nc.declare_dram_parameter("stats", [P, 4 * nt], f32, isOutput=True)

    with tile.TileContext(nc) as tc:
        with (
            tc.tile_pool(name="io", bufs=io_bufs) as io_pool,
            tc.tile_pool(name="tmp", bufs=tmp_bufs) as tmp_pool,
            tc.tile_pool(name="accp", bufs=1) as acc_pool,
        ):
            acc = acc_pool.tile([P, 4 * nt], f32)
            for t in [t for _ in range(reps) for t in range(nt)]:
                # gt+pred loads on the SP (HWDGE) sequencer, mask on Pool
                # (SWDGE) so no single sequencer serializes the DMA stream.
                gt_t = io_pool.tile([P, f], f32, tag="gt")
                nc.sync.dma_start(gt_t[:], gtD[:, bass.ts(t, f)])
                mask_t = io_pool.tile([P, f], f32, tag="mask")
                nc.gpsimd.dma_start(mask_t[:], maskD[:, bass.ts(t, f)])
                pred_t = io_pool.tile([P, f], f32, tag="pred")
                nc.sync.dma_start(pred_t[:], predD[:, bass.ts(t, f)])

                l1 = tmp_pool.tile([P, f], f32, tag="l1")
                nc.scalar.activation(l1[:], pred_t[:], Act.Ln)
                l2 = tmp_pool.tile([P, f], f32, tag="l2")
                nc.scalar.activation(l2[:], pred_t[:], Act.Ln, bias=1.0, scale=-1.0)

                # w = gt*mask on GpSimd; its row-sum rides ScalarE's
                # accumulate port (Copy).  n = (1-gt)*mask fused on VectorE.
                w = tmp_pool.tile([P, f], f32, tag="w")
                nc.gpsimd.tensor_tensor(w[:], gt_t[:], mask_t[:], Alu.mult)
                jc = tmp_pool.tile([P, f], f32, tag="jc")
                nc.scalar.activation(jc[:], w[:], Act.Copy,
                                     accum_out=acc[:, t : t + 1])
                n = tmp_pool.tile([P, f], f32, tag="n")
                nc.vector.affine_mul_reduce(
                    out=n[:], accum_out=acc[:, nt + t : nt + t + 1],
                    in0=gt_t[:], in1=mask_t[:], scale=-1.0, bias=1.0,
                )
                ja = tmp_pool.tile([P, f], f32, tag="junk")
                nc.vector.affine_mul_reduce(
                    out=ja[:], accum_out=acc[:, 2 * nt + t : 2 * nt + t + 1],
                    in0=w[:], in1=l1[:], scale=1.0, bias=0.0,
                )
                jb = tmp_pool.tile([P, f], f32, tag="junk")
                nc.vector.affine_mul_reduce(
                    out=jb[:], accum_out=acc[:, 3 * nt + t : 3 * nt + t + 1],
                    in0=n[:], in1=l2[:], scale=1.0, bias=0.0,
                )
            nc.sync.dma_start(outD[:], acc[:])
    nc.finalize()

    _NC_CACHE[key] = nc
    return nc


def _final_scalar(sw, sn, sa, sb, pred=None, gt=None, mask=None):
    """Host-side merge of the global sums into the balance loss."""
    pos_count = sw
    neg_total = sn
    pos_loss_sum = -sa
    neg_count = min(neg_total, NEG_RATIO * pos_count)
    if neg_count >= neg_total:
        topk_sum = -sb
    else:
        # exact OHEM fallback (never triggered for the shipped distribution):
        # sum of the k hardest negatives, ties split exactly like a sort.
        k = int(neg_count)
        p = np.asarray(pred, dtype=np.float32).ravel()
        g = np.asarray(gt, dtype=np.float32).ravel()
        m = np.asarray(mask, dtype=np.float32).ravel()
        neg_loss = (1.0 - g) * m * (-np.log1p(-p.astype(np.float64)))
        if k <= 0:
            topk_sum = 0.0
        else:
            part = np.partition(neg_loss, neg_loss.size - k)
            topk_sum = float(part[neg_loss.size - k :].sum())
    if neg_count > 0:
        out = (pos_loss_sum + topk_sum) / (pos_count + neg_count + EPS)
    else:
        out = pos_loss_sum / (pos_count + EPS)
    return np.asarray(out, dtype=np.float32).reshape(())


def run_device(pred, gt, mask, trace=False, reps=1, nt=None, io_bufs=None,
               tmp_bufs=None, **run_kwargs):
    """Shard, run the Bass kernel on 8 cores, return (sums, raw results)."""
    _ensure_concourse()
    from concourse.bass_utils import run_bass_kernel_spmd

    nt = NT if nt is None else nt
    nc = _build_nc(reps, nt=nt, io_bufs=io_bufs, tmp_bufs=tmp_bufs)
    shards = []
    for a in (pred, gt, mask):
        arr = np.ascontiguousarray(np.asarray(a, dtype=np.float32)).reshape(
            N_CORES, P, W
        )
        shards.append(arr)
    in_maps = [
        {"pred": shards[0][i], "gt": shards[1][i], "mask": shards[2][i]}
        for i in range(N_CORES)
    ]
    res = run_bass_kernel_spmd(nc, in_maps, list(range(N_CORES)), trace=trace,
                               **run_kwargs)
    stats = np.stack([np.asarray(r["stats"], dtype=np.float64) for r in res.results])
    # stats: [cores, P, 4*NT]
    s = stats.sum(axis=(0, 1))
    sw = s[0:nt].sum()
    sn = s[nt : 2 * nt].sum()
    sa = s[2 * nt : 3 * nt].sum()
    sb = s[3 * nt : 4 * nt].sum()
    return (sw, sn, sa, sb), res


def kernel(pred, gt, mask):
    pred = np.asarray(pred, dtype=np.float32)
    gt = np.asarray(gt, dtype=np.float32)
    mask = np.asarray(mask, dtype=np.float32)
    if pred.shape != FULL_SHAPE:
        # defensive pure-host path for non-conforming shapes
        p64 = pred.astype(np.float64)
        sw = float((gt * mask).sum(dtype=np.float64))
        sn = float(((1.0 - gt) * mask).sum(dtype=np.float64))
        sa = float((gt * mask * np.log(p64)).sum())
        sb = float(((1.0 - gt) * mask * np.log1p(-p64)).sum())
        return _final_scalar(sw, sn, sa, sb, pred, gt, mask)
    (sw, sn, sa, sb), _ = run_device(pred, gt, mask)
    return _final_scalar(sw, sn, sa, sb, pred, gt, mask)


# revision 9
# speedup vs baseline: 1.0658x; 1.0658x over previous
"""BalanceLoss (BCE + OHEM top-k negatives) on 8 trn2 NeuronCores.

Strategy
--------
Data-parallel: the 32x1x640x640 inputs are flattened and split into 8 equal
shards (one per core).  Each core computes four partial sums over its shard:

    sw = sum(gt * mask)                      (positive count)
    sn = sum((1 - gt) * mask)                (negative count)
    sa = sum(gt * mask * ln(pred))           (-pos_loss_sum)
    sb = sum((1 - gt) * mask * ln(1 - pred)) (-neg_loss_sum over ALL negatives)

On the host the shards are merged.  The OHEM top-k reduces to the full
negative sum whenever k = min(sn, 3*sw) == sn (all negatives kept), which is
the regime for this data distribution; an exact host fallback handles k < sn.

Per-core kernel: ScalarE does the two Ln passes (Ln(pred), Ln(1-pred) via the
free affine scale=-1/bias=1), VectorE does four fused
tensor_tensor_reduce passes (product + free-dim reduction in one
instruction), everything overlapped with the HBM DMA stream.
"""

import os
import sys

import numpy as np

# ---------------------------------------------------------------- constants
FULL_SHAPE = (32, 1, 640, 640)
TOT = 32 * 640 * 640          # 13_107_200 elements
N_CORES = 8
PER_CORE = TOT // N_CORES     # 1_638_400
P = 128                       # SBUF partitions
W = PER_CORE // P             # 12_800 free-dim elements per partition
NT = 8                        # tiles per core
F = W // NT                   # 1_600 free-dim elements per tile
IO_BUFS = 5
TMP_BUFS = 3
NEG_RATIO = 3.0
EPS = 1e-6

_CONCOURSE_PATHS = ("/opt/trn_rl_repo", "/root/.axon_site/_ro/trn_rl_repo")


def _ensure_concourse():
    try:
        import concourse.bass  # noqa: F401
    except ImportError:
        for p in _CONCOURSE_PATHS:
            if os.path.isdir(p) and p not in sys.path:
                sys.path.insert(0, p)
        import concourse.bass  # noqa: F401


_NC_CACHE = {}


def _build_nc(reps=1, nt=None, io_bufs=None, tmp_bufs=None):
    """Build the per-core Bass program (same program on every core).

    reps > 1 unrolls the whole tile loop `reps` times inside one NEFF for
    benchmarking; the fused accum writes are idempotent so results are
    unchanged."""
    nt = NT if nt is None else nt
    io_bufs = IO_BUFS if io_bufs is None else io_bufs
    tmp_bufs = TMP_BUFS if tmp_bufs is None else tmp_bufs
    f = W // nt
    assert W % nt == 0
    key = (reps, nt, io_bufs, tmp_bufs)
    if key in _NC_CACHE:
        return _NC_CACHE[key]
    _ensure_concourse()
    import concourse.bacc as bacc
    import concourse.bass as bass
    import concourse.mybir as mybir
    import concourse.tile as tile

    f32 = mybir.dt.float32
    Act = mybir.ActivationFunctionType
    Alu = mybir.AluOpType

    nc = bacc.Bacc(None, target_bir_lowering=False)
    predD = nc.declare_dram_parameter("pred", [P, W], f32, isOutput=False)
    gtD = nc.declare_dram_parameter("gt", [P, W], f32, isOutput=False)
    maskD = nc.declare_dram_parameter("mask", [P, W], f32, isOutput=False)
    # stats columns: [0:NT]=sum(w) [NT:2NT]=sum(n) [2NT:3NT]=sum(w*l1) [3NT:4NT]=sum(n*l2)
    outD = nc.declare_dram_parameter("stats", [P, 4 * nt], f32, isOutput=True)

    with tile.TileContext(nc) as tc:
        with (
            tc.tile_pool(name="io", bufs=io_bufs) as io_pool,
            tc.tile_pool(name="tmp", bufs=tmp_bufs) as tmp_pool,
            tc.tile_pool(name="accp", bufs=1) as acc_pool,
        ):
            acc = acc_pool.tile([P, 4 * nt], f32)
            for t in [t for _ in range(reps) for t in range(nt)]:
                # gt+pred loads on the SP (HWDGE) sequencer, mask on Pool
                # (SWDGE) so no single sequencer serializes the DMA stream.
                # First tile: one load per sequencer so all three issue at
                # once and the compute pipeline fills ~3us sooner.
                if t == 0:
                    e_gt, e_mask, e_pred = nc.sync, nc.scalar, nc.gpsimd
                else:
                    e_gt, e_mask, e_pred = nc.sync, nc.gpsimd, nc.sync
                gt_t = io_pool.tile([P, f], f32, tag="gt")
                e_gt.dma_start(gt_t[:], gtD[:, bass.ts(t, f)])
                mask_t = io_pool.tile([P, f], f32, tag="mask")
                e_mask.dma_start(mask_t[:], maskD[:, bass.ts(t, f)])
                pred_t = io_pool.tile([P, f], f32, tag="pred")
                e_pred.dma_start(pred_t[:], predD[:, bass.ts(t, f)])

                l1 = tmp_pool.tile([P, f], f32, tag="l1")
                nc.scalar.activation(l1[:], pred_t[:], Act.Ln)
                l2 = tmp_pool.tile([P, f], f32, tag="l2")
                nc.scalar.activation(l2[:], pred_t[:], Act.Ln, bias=1.0, scale=-1.0)

                # w = gt*mask on GpSimd; its row-sum rides ScalarE's
                # accumulate port (Copy).  n = (1-gt)*mask fused on VectorE.
                w = tmp_pool.tile([P, f], f32, tag="w")
                nc.gpsimd.tensor_tensor(w[:], gt_t[:], mask_t[:], Alu.mult)
                jc = tmp_pool.tile([P, f], f32, tag="jc")
                nc.scalar.activation(jc[:], w[:], Act.Copy,
                                     accum_out=acc[:, t : t + 1])
                n = tmp_pool.tile([P, f], f32, tag="n")
                nc.vector.affine_mul_reduce(
                    out=n[:], accum_out=acc[:, nt + t : nt + t + 1],
                    in0=gt_t[:], in1=mask_t[:], scale=-1.0, bias=1.0,
                )
                ja = tmp_pool.tile([P, f], f32, tag="junk")
                nc.vector.affine_mul_reduce(
                    out=ja[:], accum_out=acc[:, 2 * nt + t : 2 * nt + t + 1],
                    in0=w[:], in1=l1[:], scale=1.0, bias=0.0,
                )
                jb = tmp_pool.tile([P, f], f32, tag="junk")
                nc.vector.affine_mul_reduce(
                    out=jb[:], accum_out=acc[:, 3 * nt + t : 3 * nt + t + 1],
                    in0=n[:], in1=l2[:], scale=1.0, bias=0.0,
                )
            nc.sync.dma_start(outD[:], acc[:])
    nc.finalize()

    _NC_CACHE[key] = nc
    return nc


def _final_scalar(sw, sn, sa, sb, pred=None, gt=None, mask=None):
    """Host-side merge of the global sums into the balance loss."""
    pos_count = sw
    neg_total = sn
    pos_loss_sum = -sa
    neg_count = min(neg_total, NEG_RATIO * pos_count)
    if neg_count >= neg_total:
        topk_sum = -sb
    else:
        # exact OHEM fallback (never triggered for the shipped distribution):
        # sum of the k hardest negatives, ties split exactly like a sort.
        k = int(neg_count)
        p = np.asarray(pred, dtype=np.float32).ravel()
        g = np.asarray(gt, dtype=np.float32).ravel()
        m = np.asarray(mask, dtype=np.float32).ravel()
        neg_loss = (1.0 - g) * m * (-np.log1p(-p.astype(np.float64)))
        if k <= 0:
            topk_sum = 0.0
        else:
            part = np.partition(neg_loss, neg_loss.size - k)
            topk_sum = float(part[neg_loss.size - k :].sum())
    if neg_count > 0:
        out = (pos_loss_sum + topk_sum) / (pos_count + neg_count + EPS)
    else:
        out = pos_loss_sum / (pos_count + EPS)
    return np.asarray(out, dtype=np.float32).reshape(())


def run_device(pred, gt, mask, trace=False, reps=1, nt=None, io_bufs=None,
               tmp_bufs=None, **run_kwargs):
    """Shard, run the Bass kernel on 8 cores, return (sums, raw results)."""
    _ensure_concourse()
    from concourse.bass_utils import run_bass_kernel_spmd

    nt = NT if nt is None else nt
    nc = _build_nc(reps, nt=nt, io_bufs=io_bufs, tmp_bufs=tmp_bufs)
    shards = []
    for a in (pred, gt, mask):
        arr = np.ascontiguousarray(np.asarray(a, dtype=np.float32)).reshape(
            N_CORES, P, W
        )
        shards.append(arr)
    in_maps = [
        {"pred": shards[0][i], "gt": shards[1][i], "mask": shards[2][i]}
        for i in range(N_CORES)
    ]
    res = run_bass_kernel_spmd(nc, in_maps, list(range(N_CORES)), trace=trace,
                               **run_kwargs)
    stats = np.stack([np.asarray(r["stats"], dtype=np.float64) for r in res.results])
    # stats: [cores, P, 4*NT]
    s = stats.sum(axis=(0, 1))
    sw = s[0:nt].sum()
    sn = s[nt : 2 * nt].sum()
    sa = s[2 * nt : 3 * nt].sum()
    sb = s[3 * nt : 4 * nt].sum()
    return (sw, sn, sa, sb), res


def kernel(pred, gt, mask):
    pred = np.asarray(pred, dtype=np.float32)
    gt = np.asarray(gt, dtype=np.float32)
    mask = np.asarray(mask, dtype=np.float32)
    if pred.shape != FULL_SHAPE:
        # defensive pure-host path for non-conforming shapes
        p64 = pred.astype(np.float64)
        sw = float((gt * mask).sum(dtype=np.float64))
        sn = float(((1.0 - gt) * mask).sum(dtype=np.float64))
        sa = float((gt * mask * np.log(p64)).sum())
        sb = float(((1.0 - gt) * mask * np.log1p(-p64)).sum())
        return _final_scalar(sw, sn, sa, sb, pred, gt, mask)
    (sw, sn, sa, sb), _ = run_device(pred, gt, mask)
    return _final_scalar(sw, sn, sa, sb, pred, gt, mask)


# revision 10
# speedup vs baseline: 1.0773x; 1.0108x over previous
"""BalanceLoss (BCE + OHEM top-k negatives) on 8 trn2 NeuronCores.

Strategy
--------
Data-parallel: the 32x1x640x640 inputs are flattened and split into 8 equal
shards (one per core).  Each core computes four partial sums over its shard:

    sw = sum(gt * mask)                      (positive count)
    sn = sum((1 - gt) * mask)                (negative count)
    sa = sum(gt * mask * ln(pred))           (-pos_loss_sum)
    sb = sum((1 - gt) * mask * ln(1 - pred)) (-neg_loss_sum over ALL negatives)

On the host the shards are merged.  The OHEM top-k reduces to the full
negative sum whenever k = min(sn, 3*sw) == sn (all negatives kept), which is
the regime for this data distribution; an exact host fallback handles k < sn.

Per-core schedule (all five engines share the work so each stays under the
~55us HBM roofline for the 19.7MB/core the kernel must stream):
  - ScalarE: both Ln passes (ln(1-pred) via the free affine scale=-1/bias=1).
  - GpSimd:  w = gt*mask products (and n = mask-w on alternating tiles).
  - PE:      sum(w) (and sum(n) on those tiles) via accumulating matmuls
             against a ones vector into one PSUM bank.
  - VectorE: the loss products as fused affine_mul_reduce (product +
             free-dim sum in a single instruction), plus n on the other tiles.
  - DMA issue is spread over the SP/Pool/ScalarE sequencers (~1us of
    sequencer occupancy per dma_start would otherwise serialize).
"""

import os
import sys

import numpy as np

# ---------------------------------------------------------------- constants
FULL_SHAPE = (32, 1, 640, 640)
TOT = 32 * 640 * 640          # 13_107_200 elements
N_CORES = 8
PER_CORE = TOT // N_CORES     # 1_638_400
P = 128                       # SBUF partitions
W = PER_CORE // P             # 12_800 free-dim elements per partition
NT = 8                        # tiles per core
F = W // NT                   # 1_600 free-dim elements per tile
IO_BUFS = 5
TMP_BUFS = 3
GP_N_TILES = (1, 3, 5, 7)     # tiles whose n-chain runs on GpSimd+PE
MMCHUNK = 512                 # PSUM bank width for the PE reductions
NEG_RATIO = 3.0
EPS = 1e-6

_CONCOURSE_PATHS = ("/opt/trn_rl_repo", "/root/.axon_site/_ro/trn_rl_repo")


def _ensure_concourse():
    try:
        import concourse.bass  # noqa: F401
    except ImportError:
        for p in _CONCOURSE_PATHS:
            if os.path.isdir(p) and p not in sys.path:
                sys.path.insert(0, p)
        import concourse.bass  # noqa: F401


_NC_CACHE = {}


def _build_nc(reps=1):
    """Build the per-core Bass program (same program on every core).

    reps > 1 unrolls the whole tile loop `reps` times inside one NEFF for
    benchmarking; accumulators are rewritten per rep so results are
    unchanged."""
    if reps in _NC_CACHE:
        return _NC_CACHE[reps]
    _ensure_concourse()
    import concourse.bacc as bacc
    import concourse.bass as bass
    import concourse.mybir as mybir
    import concourse.tile as tile

    f32 = mybir.dt.float32
    Act = mybir.ActivationFunctionType
    Alu = mybir.AluOpType

    nc = bacc.Bacc(None, target_bir_lowering=False)
    predD = nc.declare_dram_parameter("pred", [P, W], f32, isOutput=False)
    gtD = nc.declare_dram_parameter("gt", [P, W], f32, isOutput=False)
    maskD = nc.declare_dram_parameter("mask", [P, W], f32, isOutput=False)
    # stats columns: [0:NT]=sum(n) per AMR tile (0 on GP_N_TILES),
    # [NT:2NT]=sum(w*l1), [2NT:3NT]=sum(n*l2), [3NT]=sum(w) from PE,
    # [3NT+1]=sum(n) from PE (partition 0 only for the last two).
    outD = nc.declare_dram_parameter("stats", [P, 3 * NT + 2], f32, isOutput=True)

    n_w_mms = NT * ((F + MMCHUNK - 1) // MMCHUNK)
    n_n_mms = len(GP_N_TILES) * ((F + MMCHUNK - 1) // MMCHUNK)

    with tile.TileContext(nc) as tc:
        with (
            tc.tile_pool(name="io", bufs=IO_BUFS) as io_pool,
            tc.tile_pool(name="tmp", bufs=TMP_BUFS) as tmp_pool,
            tc.tile_pool(name="accp", bufs=1) as acc_pool,
            tc.tile_pool(name="ps", bufs=1, space="PSUM") as ps_pool,
        ):
            acc = acc_pool.tile([P, 3 * NT + 2], f32)
            nc.vector.memset(acc[:], 0.0)
            ones = acc_pool.tile([P, 1], f32)
            nc.gpsimd.memset(ones[:], 1.0)
            psum = ps_pool.tile([1, 2 * MMCHUNK], f32)
            for rep in range(reps):
                wm = nm = 0
                for t in range(NT):
                    sl = bass.ts(t, F)
                    # spread the first tile's loads over three sequencers so
                    # the pipeline fills as early as possible
                    if t == 0:
                        e_pred, e_gt, e_mask = nc.gpsimd, nc.sync, nc.scalar
                    else:
                        e_pred, e_gt, e_mask = nc.sync, nc.sync, nc.gpsimd
                    gt_t = io_pool.tile([P, F], f32, tag="gt")
                    e_gt.dma_start(gt_t[:], gtD[:, sl])
                    mask_t = io_pool.tile([P, F], f32, tag="mask")
                    e_mask.dma_start(mask_t[:], maskD[:, sl])
                    pred_t = io_pool.tile([P, F], f32, tag="pred")
                    e_pred.dma_start(pred_t[:], predD[:, sl])

                    l1 = tmp_pool.tile([P, F], f32, tag="l1")
                    nc.scalar.activation(l1[:], pred_t[:], Act.Ln)
                    l2 = tmp_pool.tile([P, F], f32, tag="l2")
                    nc.scalar.activation(l2[:], pred_t[:], Act.Ln,
                                         bias=1.0, scale=-1.0)

                    w = tmp_pool.tile([P, F], f32, tag="w")
                    nc.gpsimd.tensor_tensor(w[:], gt_t[:], mask_t[:], Alu.mult)
                    for c in range(0, F, MMCHUNK):
                        cw = min(MMCHUNK, F - c)
                        nc.tensor.matmul(
                            psum[0:1, 0:cw], ones[:, 0:1], w[:, c : c + cw],
                            start=(wm == 0), stop=(wm == n_w_mms - 1),
                            skip_group_check=True,
                        )
                        wm += 1
                    n = tmp_pool.tile([P, F], f32, tag="n")
                    if t in GP_N_TILES:
                        nc.gpsimd.tensor_tensor(n[:], mask_t[:], w[:], Alu.subtract)
                        for c in range(0, F, MMCHUNK):
                            cw = min(MMCHUNK, F - c)
                            nc.tensor.matmul(
                                psum[0:1, MMCHUNK : MMCHUNK + cw], ones[:, 0:1],
                                n[:, c : c + cw],
                                start=(nm == 0), stop=(nm == n_n_mms - 1),
                                skip_group_check=True,
                            )
                            nm += 1
                    else:
                        nc.vector.affine_mul_reduce(
                            out=n[:], accum_out=acc[:, t : t + 1],
                            in0=gt_t[:], in1=mask_t[:], scale=-1.0, bias=1.0,
                        )
                    ja = tmp_pool.tile([P, F], f32, tag="junk")
                    nc.vector.affine_mul_reduce(
                        out=ja[:], accum_out=acc[:, NT + t : NT + t + 1],
                        in0=w[:], in1=l1[:], scale=1.0, bias=0.0,
                    )
                    jb = tmp_pool.tile([P, F], f32, tag="junk")
                    nc.vector.affine_mul_reduce(
                        out=jb[:], accum_out=acc[:, 2 * NT + t : 2 * NT + t + 1],
                        in0=n[:], in1=l2[:], scale=1.0, bias=0.0,
                    )
                # fold the PSUM accumulators into two acc columns (partition 0)
                nc.vector.tensor_reduce(
                    acc[0:1, 3 * NT : 3 * NT + 1], psum[0:1, 0:MMCHUNK],
                    axis=mybir.AxisListType.X, op=Alu.add)
                nc.vector.tensor_reduce(
                    acc[0:1, 3 * NT + 1 : 3 * NT + 2],
                    psum[0:1, MMCHUNK : 2 * MMCHUNK],
                    axis=mybir.AxisListType.X, op=Alu.add)
            nc.sync.dma_start(outD[:], acc[:])
    nc.finalize()

    _NC_CACHE[reps] = nc
    return nc


def _final_scalar(sw, sn, sa, sb, pred=None, gt=None, mask=None):
    """Host-side merge of the global sums into the balance loss."""
    pos_count = sw
    neg_total = sn
    pos_loss_sum = -sa
    neg_count = min(neg_total, NEG_RATIO * pos_count)
    if neg_count >= neg_total:
        topk_sum = -sb
    else:
        # exact OHEM fallback (never triggered for the shipped distribution):
        # sum of the k hardest negatives, ties split exactly like a sort.
        k = int(neg_count)
        p = np.asarray(pred, dtype=np.float32).ravel()
        g = np.asarray(gt, dtype=np.float32).ravel()
        m = np.asarray(mask, dtype=np.float32).ravel()
        neg_loss = (1.0 - g) * m * (-np.log1p(-p.astype(np.float64)))
        if k <= 0:
            topk_sum = 0.0
        else:
            part = np.partition(neg_loss, neg_loss.size - k)
            topk_sum = float(part[neg_loss.size - k :].sum())
    if neg_count > 0:
        out = (pos_loss_sum + topk_sum) / (pos_count + neg_count + EPS)
    else:
        out = pos_loss_sum / (pos_count + EPS)
    return np.asarray(out, dtype=np.float32).reshape(())


def run_device(pred, gt, mask, trace=False, reps=1, **run_kwargs):
    """Shard, run the Bass kernel on 8 cores, return (sums, raw results)."""
    _ensure_concourse()
    from concourse.bass_utils import run_bass_kernel_spmd

    nc = _build_nc(reps)
    shards = []
    for a in (pred, gt, mask):
        arr = np.ascontiguousarray(np.asarray(a, dtype=np.float32)).reshape(
            N_CORES, P, W
        )
        shards.append(arr)
    in_maps = [
        {"pred": shards[0][i], "gt": shards[1][i], "mask": shards[2][i]}
        for i in range(N_CORES)
    ]
    res = run_bass_kernel_spmd(nc, in_maps, list(range(N_CORES)), trace=trace,
                               **run_kwargs)
    stats = np.stack([np.asarray(r["stats"], dtype=np.float64) for r in res.results])
    # stats: [cores, P, 3*NT+2]; sum over cores and partitions
    s = stats.sum(axis=(0, 1))
    sw = s[3 * NT]
    sn = s[0:NT].sum() + s[3 * NT + 1]
    sa = s[NT : 2 * NT].sum()
    sb = s[2 * NT : 3 * NT].sum()
    return (sw, sn, sa, sb), res


def kernel(pred, gt, mask):
    pred = np.asarray(pred, dtype=np.float32)
    gt = np.asarray(gt, dtype=np.float32)
    mask = np.asarray(mask, dtype=np.float32)
    if pred.shape != FULL_SHAPE:
        # defensive pure-host path for non-conforming shapes
        p64 = pred.astype(np.float64)
        sw = float((gt * mask).sum(dtype=np.float64))
        sn = float(((1.0 - gt) * mask).sum(dtype=np.float64))
        sa = float((gt * mask * np.log(p64)).sum())
        sb = float(((1.0 - gt) * mask * np.log1p(-p64)).sum())
        return _final_scalar(sw, sn, sa, sb, pred, gt, mask)
    (sw, sn, sa, sb), _ = run_device(pred, gt, mask)
    return _final_scalar(sw, sn, sa, sb, pred, gt, mask)


# revision 11
# speedup vs baseline: 1.1003x; 1.0213x over previous
"""BalanceLoss (BCE + OHEM top-k negatives) on 8 trn2 NeuronCores.

Strategy
--------
Data-parallel: the 32x1x640x640 inputs are flattened and split into 8 equal
shards (one per core).  Each core computes four partial sums over its shard:

    sw = sum(gt * mask)                      (positive count)
    sn = sum((1 - gt) * mask)                (negative count)
    sa = sum(gt * mask * ln(pred))           (-pos_loss_sum)
    sb = sum((1 - gt) * mask * ln(1 - pred)) (-neg_loss_sum over ALL negatives)

On the host the shards are merged.  The OHEM top-k reduces to the full
negative sum whenever k = min(sn, 3*sw) == sn (all negatives kept), which is
the regime for this data distribution; an exact host fallback handles k < sn.

Per-core schedule (all five engines share the work so each stays under the
~55us HBM roofline for the 19.7MB/core the kernel must stream):
  - ScalarE: both Ln passes (ln(1-pred) via the free affine scale=-1/bias=1).
  - GpSimd:  w = gt*mask products (and n = mask-w on alternating tiles).
  - PE:      sum(w) (and sum(n) on those tiles) via accumulating matmuls
             against a ones vector into one PSUM bank.
  - VectorE: the loss products as fused affine_mul_reduce (product +
             free-dim sum in a single instruction), plus n on the other tiles.
  - DMA issue is spread over the SP/Pool/ScalarE sequencers (~1us of
    sequencer occupancy per dma_start would otherwise serialize).
"""

import os
import sys

import numpy as np

# ---------------------------------------------------------------- constants
FULL_SHAPE = (32, 1, 640, 640)
TOT = 32 * 640 * 640          # 13_107_200 elements
N_CORES = 8
PER_CORE = TOT // N_CORES     # 1_638_400
P = 128                       # SBUF partitions
W = PER_CORE // P             # 12_800 free-dim elements per partition
NT = 16                       # compute tiles per core
F = W // NT                   # 800 free-dim elements per tile
DMA_GROUP = 2                 # one [P, F*DMA_GROUP] load feeds 2 compute tiles
IO_BUFS = 4
TMP_BUFS = 4
GP_N_TILES = tuple(range(1, 16, 2))  # tiles whose n-chain runs on GpSimd+PE
MMCHUNK = 512                 # PSUM bank width for the PE reductions
NEG_RATIO = 3.0
EPS = 1e-6

_CONCOURSE_PATHS = ("/opt/trn_rl_repo", "/root/.axon_site/_ro/trn_rl_repo")


def _ensure_concourse():
    try:
        import concourse.bass  # noqa: F401
    except ImportError:
        for p in _CONCOURSE_PATHS:
            if os.path.isdir(p) and p not in sys.path:
                sys.path.insert(0, p)
        import concourse.bass  # noqa: F401


_NC_CACHE = {}


def _build_nc(reps=1):
    """Build the per-core Bass program (same program on every core).

    reps > 1 unrolls the whole tile loop `reps` times inside one NEFF for
    benchmarking; accumulators are rewritten per rep so results are
    unchanged."""
    if reps in _NC_CACHE:
        return _NC_CACHE[reps]
    _ensure_concourse()
    import concourse.bacc as bacc
    import concourse.bass as bass
    import concourse.mybir as mybir
    import concourse.tile as tile

    f32 = mybir.dt.float32
    Act = mybir.ActivationFunctionType
    Alu = mybir.AluOpType

    nc = bacc.Bacc(None, target_bir_lowering=False)
    predD = nc.declare_dram_parameter("pred", [P, W], f32, isOutput=False)
    gtD = nc.declare_dram_parameter("gt", [P, W], f32, isOutput=False)
    maskD = nc.declare_dram_parameter("mask", [P, W], f32, isOutput=False)
    # stats columns: [0:NT]=sum(n) per AMR tile (0 on GP_N_TILES),
    # [NT:2NT]=sum(w*l1), [2NT:3NT]=sum(n*l2), [3NT]=sum(w) from PE,
    # [3NT+1]=sum(n) from PE (partition 0 only for the last two).
    outD = nc.declare_dram_parameter("stats", [P, 3 * NT + 2], f32, isOutput=True)

    n_w_mms = NT * ((F + MMCHUNK - 1) // MMCHUNK)
    n_n_mms = len(GP_N_TILES) * ((F + MMCHUNK - 1) // MMCHUNK)

    with tile.TileContext(nc) as tc:
        with (
            tc.tile_pool(name="io", bufs=IO_BUFS) as io_pool,
            tc.tile_pool(name="tmp", bufs=TMP_BUFS) as tmp_pool,
            tc.tile_pool(name="accp", bufs=1) as acc_pool,
            tc.tile_pool(name="ps", bufs=1, space="PSUM") as ps_pool,
        ):
            acc = acc_pool.tile([P, 3 * NT + 2], f32)
            nc.vector.memset(acc[:], 0.0)
            ones = acc_pool.tile([P, 1], f32)
            nc.gpsimd.memset(ones[:], 1.0)
            psum = ps_pool.tile([1, 2 * MMCHUNK], f32)
            FD = F * DMA_GROUP
            for rep in range(reps):
                wm = nm = 0
                gt_g = mask_g = pred_g = None
                for t in range(NT):
                    if t % DMA_GROUP == 0:
                        sl = slice(t * F, t * F + FD)
                        # spread the first loads over three sequencers so the
                        # pipeline fills as early as possible
                        if t == 0:
                            e_pred, e_gt, e_mask = nc.gpsimd, nc.sync, nc.scalar
                        else:
                            e_pred, e_gt, e_mask = nc.sync, nc.sync, nc.gpsimd
                        gt_g = io_pool.tile([P, FD], f32, tag="gt")
                        e_gt.dma_start(gt_g[:], gtD[:, sl])
                        mask_g = io_pool.tile([P, FD], f32, tag="mask")
                        e_mask.dma_start(mask_g[:], maskD[:, sl])
                        pred_g = io_pool.tile([P, FD], f32, tag="pred")
                        e_pred.dma_start(pred_g[:], predD[:, sl])
                    h = (t % DMA_GROUP) * F
                    gt_t = gt_g[:, h : h + F]
                    mask_t = mask_g[:, h : h + F]
                    pred_t = pred_g[:, h : h + F]

                    l1 = tmp_pool.tile([P, F], f32, tag="l1")
                    nc.scalar.activation(l1[:], pred_t, Act.Ln)
                    l2 = tmp_pool.tile([P, F], f32, tag="l2")
                    nc.scalar.activation(l2[:], pred_t, Act.Ln,
                                         bias=1.0, scale=-1.0)

                    w = tmp_pool.tile([P, F], f32, tag="w")
                    nc.gpsimd.tensor_tensor(w[:], gt_t, mask_t, Alu.mult)
                    for c in range(0, F, MMCHUNK):
                        cw = min(MMCHUNK, F - c)
                        nc.tensor.matmul(
                            psum[0:1, 0:cw], ones[:, 0:1], w[:, c : c + cw],
                            start=(wm == 0), stop=(wm == n_w_mms - 1),
                            skip_group_check=True,
                        )
                        wm += 1
                    n = tmp_pool.tile([P, F], f32, tag="n")
                    if t in GP_N_TILES:
                        nc.gpsimd.tensor_tensor(n[:], mask_t, w[:], Alu.subtract)
                        for c in range(0, F, MMCHUNK):
                            cw = min(MMCHUNK, F - c)
                            nc.tensor.matmul(
                                psum[0:1, MMCHUNK : MMCHUNK + cw], ones[:, 0:1],
                                n[:, c : c + cw],
                                start=(nm == 0), stop=(nm == n_n_mms - 1),
                                skip_group_check=True,
                            )
                            nm += 1
                    else:
                        nc.vector.affine_mul_reduce(
                            out=n[:], accum_out=acc[:, t : t + 1],
                            in0=gt_t, in1=mask_t, scale=-1.0, bias=1.0,
                        )
                    ja = tmp_pool.tile([P, F], f32, tag="junk")
                    nc.vector.affine_mul_reduce(
                        out=ja[:], accum_out=acc[:, NT + t : NT + t + 1],
                        in0=w[:], in1=l1[:], scale=1.0, bias=0.0,
                    )
                    jb = tmp_pool.tile([P, F], f32, tag="junk")
                    nc.vector.affine_mul_reduce(
                        out=jb[:], accum_out=acc[:, 2 * NT + t : 2 * NT + t + 1],
                        in0=n[:], in1=l2[:], scale=1.0, bias=0.0,
                    )
                # fold the PSUM accumulators into two acc columns (partition 0)
                nc.vector.tensor_reduce(
                    acc[0:1, 3 * NT : 3 * NT + 1], psum[0:1, 0:MMCHUNK],
                    axis=mybir.AxisListType.X, op=Alu.add)
                nc.vector.tensor_reduce(
                    acc[0:1, 3 * NT + 1 : 3 * NT + 2],
                    psum[0:1, MMCHUNK : 2 * MMCHUNK],
                    axis=mybir.AxisListType.X, op=Alu.add)
            nc.sync.dma_start(outD[:], acc[:])
    nc.finalize()

    _NC_CACHE[reps] = nc
    return nc


def _final_scalar(sw, sn, sa, sb, pred=None, gt=None, mask=None):
    """Host-side merge of the global sums into the balance loss."""
    pos_count = sw
    neg_total = sn
    pos_loss_sum = -sa
    neg_count = min(neg_total, NEG_RATIO * pos_count)
    if neg_count >= neg_total:
        topk_sum = -sb
    else:
        # exact OHEM fallback (never triggered for the shipped distribution):
        # sum of the k hardest negatives, ties split exactly like a sort.
        k = int(neg_count)
        p = np.asarray(pred, dtype=np.float32).ravel()
        g = np.asarray(gt, dtype=np.float32).ravel()
        m = np.asarray(mask, dtype=np.float32).ravel()
        neg_loss = (1.0 - g) * m * (-np.log1p(-p.astype(np.float64)))
        if k <= 0:
            topk_sum = 0.0
        else:
            part = np.partition(neg_loss, neg_loss.size - k)
            topk_sum = float(part[neg_loss.size - k :].sum())
    if neg_count > 0:
        out = (pos_loss_sum + topk_sum) / (pos_count + neg_count + EPS)
    else:
        out = pos_loss_sum / (pos_count + EPS)
    return np.asarray(out, dtype=np.float32).reshape(())


def run_device(pred, gt, mask, trace=False, reps=1, **run_kwargs):
    """Shard, run the Bass kernel on 8 cores, return (sums, raw results)."""
    _ensure_concourse()
    from concourse.bass_utils import run_bass_kernel_spmd

    nc = _build_nc(reps)
    shards = []
    for a in (pred, gt, mask):
        arr = np.ascontiguousarray(np.asarray(a, dtype=np.float32)).reshape(
            N_CORES, P, W
        )
        shards.append(arr)
    in_maps = [
        {"pred": shards[0][i], "gt": shards[1][i], "mask": shards[2][i]}
        for i in range(N_CORES)
    ]
    res = run_bass_kernel_spmd(nc, in_maps, list(range(N_CORES)), trace=trace,
                               **run_kwargs)
    stats = np.stack([np.asarray(r["stats"], dtype=np.float64) for r in res.results])
    # stats: [cores, P, 3*NT+2]; sum over cores and partitions
    s = stats.sum(axis=(0, 1))
    sw = s[3 * NT]
    sn = s[0:NT].sum() + s[3 * NT + 1]
    sa = s[NT : 2 * NT].sum()
    sb = s[2 * NT : 3 * NT].sum()
    return (sw, sn, sa, sb), res


def kernel(pred, gt, mask):
    pred = np.asarray(pred, dtype=np.float32)
    gt = np.asarray(gt, dtype=np.float32)
    mask = np.asarray(mask, dtype=np.float32)
    if pred.shape != FULL_SHAPE:
        # defensive pure-host path for non-conforming shapes
        p64 = pred.astype(np.float64)
        sw = float((gt * mask).sum(dtype=np.float64))
        sn = float(((1.0 - gt) * mask).sum(dtype=np.float64))
        sa = float((gt * mask * np.log(p64)).sum())
        sb = float(((1.0 - gt) * mask * np.log1p(-p64)).sum())
        return _final_scalar(sw, sn, sa, sb, pred, gt, mask)
    (sw, sn, sa, sb), _ = run_device(pred, gt, mask)
    return _final_scalar(sw, sn, sa, sb, pred, gt, mask)
